# revision 1
# baseline (speedup 1.0000x reference)
"""NexusNet Trainium2 kernel (8-core SPMD, Bass/Tile) — v2.

Two SPMD launches, graph/data parallel per the sharding hint:

L1 (dst-partition): each core owns M/8 nexus rows. Host pre-gathers the
   x[src] rows for each core's edges into tile-ordered slabs (plain
   streaming DMA on device, no indexed gather). Per 128-row nexus window:
   feature-major segment sums accumulate in PSUM via per-tile slice
   matmuls against uploaded one-hot matrices; the nexus MLP runs
   feature-major batched over 4-window slabs; per-nexus-row linear
   precomputes an = W1n_edge^T n + b1_edge and z = W1n_node^T n are
   transposed back to row-major on the PE and written as packed
   [an | z] rows (640 bf16 = 1280B).

Host concatenates the 8 andz slices (index-space stitch only).

L2 (src-partition): each core owns N/8 plane nodes + their edges.
   Per group of 8 node blocks: ax = x W1x_edge and axnT = (x W1x_node)^T
   are computed on the fly from uploaded xT slabs (SBUF only, no DRAM
   round trip). Edge tiles dma_gather their [an|z][dst] rows; axe is
   selected from ax via uploaded one-hot-transpose matmuls with the
   gathered an added through an identity matmul (all in PSUM); tanh /
   softmax / msg run batched over up to W2B tiles; agg accumulates
   feature-major via 3 slice matmuls per tile into per-block PSUM seeded
   with axnT (+node bias), so u1 = tanh(PSUM) needs no transposes and
   u2 = tanh(u1 W2 + b2) is a plain feature-major matmul. Inverse
   degree (mean aggregation) is folded into the per-edge softmax weight
   via a per-slot table. Output is written bf16 feature-major.
"""

import numpy as np

from concourse import bacc, bass, mybir, tile
from concourse.bass_utils import run_bass_kernel_spmd

DT = mybir.dt
BF = DT.bfloat16
F8 = DT.float8e3
F32 = DT.float32
I16 = DT.int16
AF = mybir.ActivationFunctionType
OP = mybir.AluOpType


def ceil_div(a, b):
    return (a + b - 1) // b


class Cfg:
    def __init__(self, N=80000, M=40000, E=120000, ncores=8):
        self.N, self.M, self.E = N, M, E
        self.C, self.F, self.S, self.EF, self.P = 5, 64, 64, 64, 3
        self.RB = 384           # padded x row elems (C*F=320 -> 384; 768B)
        self.RZ = 640           # [an|z] row elems (1280B)
        self.NC = ncores
        assert N % ncores == 0 and M % ncores == 0
        self.NK = N // ncores               # plane nodes per core
        self.MK = M // ncores               # nexus rows per core
        self.NW = ceil_div(self.MK, 128)    # L1 windows per core
        self.MKP = self.NW * 128
        self.NB = ceil_div(self.NK, 512) * 4   # L2 blocks (pad to 4-multiples)
        self.NKP = self.NB * 128
        self.NKP512 = self.NB * 128          # same padding (512-mult)
        self.SLW = 4                         # L1 windows per MLP slab
        self.NSL1 = ceil_div(self.NW, self.SLW)
        self.HROWS = ncores * self.MKP       # stitched andz rows
        self.DCH = (self.HROWS if self.HROWS <= 32767
                    else ceil_div(self.HROWS, 2))
        self.NH = ceil_div(self.HROWS, self.DCH)
        assert self.DCH <= 32767
        self.BG = min(8, self.NB)            # L2 blocks per gather group
        self.SG = 4                          # L2 blocks per agg sub-group
        self.W2B = 5                         # L2 edge tiles per softmax batch
        self.UB = 4                          # L2 blocks per u2 slab
        self.NUSL = self.NB // self.UB
        assert self.NB % self.UB == 0 and self.NB % self.BG == 0
        assert self.BG % self.SG == 0 and self.BG % self.UB == 0
        self.GRP = [(0, 1), (2, 3), (4,)]    # class pair groups
        self.GP = [128, 128, 64]


def wrap_idx(idx):
    """[n] int array -> [128, n//16] int16 wrapped+replicated layout."""
    n = len(idx)
    assert n % 16 == 0
    w16 = np.asarray(idx, np.int16).reshape(n // 16, 16).T.copy()
    return np.tile(w16, (8, 1))


def blkdiag(mats, rows=None, cols=None):
    rs = sum(m.shape[0] for m in mats)
    cs = sum(m.shape[1] for m in mats)
    out = np.zeros((rows or rs, cols or cs), np.float32)
    r = c = 0
    for m in mats:
        out[r:r + m.shape[0], c:c + m.shape[1]] = m
        r += m.shape[0]
        c += m.shape[1]
    return out


def to_bf16(x):
    import ml_dtypes
    return np.asarray(x).astype(ml_dtypes.bfloat16)


def to_f8(x):
    import ml_dtypes
    return np.asarray(x).astype(ml_dtypes.float8_e3m4)


def _wdt(arr):
    return BF if arr.dtype != np.float32 else F32


# ----------------------------------------------------------------------------
# Host preprocessing
# ----------------------------------------------------------------------------

class Prep:
    pass


def host_prep(cfg, inputs):
    c = cfg
    pr = Prep()
    planes = "uvy"
    bf = to_bf16(np.zeros(1)).dtype

    xs = [np.asarray(inputs[f"x_{p}"], np.float32).reshape(c.N, c.C * c.F)
          for p in planes]
    edges = [np.asarray(inputs[f"edge_{p}"], np.int64) for p in planes]

    # xT slabs for L2 prologue: [128, 3, NKP512] per (core, plane)
    pr.xt = [[None] * c.P for _ in range(c.NC)]
    for k in range(c.NC):
        for p in range(c.P):
            sl = np.zeros((c.NKP512, 384), np.float32)
            sl[:c.NK, :320] = xs[p][k * c.NK:(k + 1) * c.NK]
            xt = sl.T.reshape(3, 128, c.NKP512).transpose(1, 0, 2)
            pr.xt[k][p] = to_bf16(np.ascontiguousarray(xt))

    # ---------------- L1: edges grouped by dst window ----------------------
    # per (core, plane): tiles ordered w-major; host pre-gathers x rows.
    pr.T1 = [[0] * c.NW for _ in range(c.P)]
    l1 = [[None] * c.P for _ in range(c.NC)]
    for p in range(c.P):
        src, dst = edges[p][0], edges[p][1]
        owner = dst // c.MK
        for k in range(c.NC):
            sel = owner == k
            s = src[sel]
            d = dst[sel] - k * c.MK
            w = d // 128
            order = np.argsort(w, kind="stable")
            l1[k][p] = (s[order], (d - w * 128)[order], w[order])
        for w in range(c.NW):
            mx = max(int((l1[k][p][2] == w).sum()) for k in range(c.NC))
            pr.T1[p][w] = max(ceil_div(mx, 128), 1)
    pr.T1tot = [sum(pr.T1[p]) for p in range(c.P)]

    pr.xg = [[None] * c.P for _ in range(c.NC)]
    pr.oh1 = [[None] * c.P for _ in range(c.NC)]
    for k in range(c.NC):
        for p in range(c.P):
            tt = pr.T1tot[p]
            f8 = to_f8(np.zeros(1)).dtype
            xg = np.zeros((tt * 128, 320), f8)
            oh = np.zeros((128, tt, 128), f8)
            s_all, dl_all, w_all = l1[k][p]
            t0 = 0
            for w in range(c.NW):
                m = w_all == w
                s, dl = s_all[m], dl_all[m]
                n = len(s)
                xg[t0 * 128:t0 * 128 + n] = to_f8(
                    xs[p][s].astype(np.float32))
                ti = t0 + np.arange(n) // 128
                sl = np.arange(n) % 128
                oh[sl, ti, dl] = 1.0
                t0 += pr.T1[p][w]
            pr.xg[k][p] = np.ascontiguousarray(
                xg.reshape(tt, 128, 320).transpose(1, 0, 2))
            pr.oh1[k][p] = oh

    # ---------------- L2: edges grouped by (src block, dst h-chunk) --------
    l2_lists = [[[[None] * c.NH for _ in range(c.NB)] for _ in range(c.P)]
                for _ in range(c.NC)]
    for p in range(c.P):
        src, dst = edges[p][0], edges[p][1]
        owner = src // c.NK
        drow = (dst // c.MK) * c.MKP + (dst % c.MK)
        for k in range(c.NC):
            sel = owner == k
            s = src[sel] - k * c.NK
            dr = drow[sel]
            deg = np.bincount(s, minlength=c.NKP).astype(np.float32)
            ic = 1.0 / np.maximum(deg, 1.0)
            b = s // 128
            h = dr // c.DCH
            for bb in range(c.NB):
                for hh in range(c.NH):
                    m = (b == bb) & (h == hh)
                    l2_lists[k][p][bb][hh] = (s[m] - bb * 128,
                                              dr[m] - hh * c.DCH,
                                              ic[s[m]])

    pr.T2 = [[[0] * c.NH for _ in range(c.NB)] for _ in range(c.P)]
    for p in range(c.P):
        for b in range(c.NB):
            for h in range(c.NH):
                mx = max(len(l2_lists[k][p][b][h][0]) for k in range(c.NC))
                pr.T2[p][b][h] = ceil_div(mx, 128)

    # global tile order: b asc, h asc; per-h dense order for gathers
    pr.tiles2 = []          # per p: list of (b, h, posh)
    pr.htiles2 = []
    for p in range(c.P):
        hpos = [0] * c.NH
        tl = []
        for b in range(c.NB):
            for h in range(c.NH):
                for t in range(pr.T2[p][b][h]):
                    tl.append((b, h, hpos[h]))
                    hpos[h] += 1
        pr.tiles2.append(tl)
        pr.htiles2.append(hpos)

    pr.NBG = ceil_div(c.NB, c.BG)
    # per (p, h, g): (start pos in h-dense order, ntiles)
    pr.seg2d = [[[None] * pr.NBG for _ in range(c.NH)] for _ in range(c.P)]
    for p in range(c.P):
        hseen = [0] * c.NH
        for g in range(pr.NBG):
            b0, b1 = g * c.BG, min((g + 1) * c.BG, c.NB)
            for h in range(c.NH):
                nh = sum(pr.T2[p][b][h] for b in range(b0, b1))
                pr.seg2d[p][h][g] = (hseen[h], nh)
                hseen[h] += nh

    # h-dense column offsets for the icvt table (batch-consecutive layout)
    pr.hoff = [[0] * (c.NH + 1) for _ in range(c.P)]
    for p in range(c.P):
        for h in range(c.NH):
            pr.hoff[p][h + 1] = pr.hoff[p][h] + pr.htiles2[p][h]

    pr.gidx2d = [[[None] * c.NH for _ in range(c.P)] for _ in range(c.NC)]
    pr.oh2 = [[None] * c.P for _ in range(c.NC)]
    pr.icvt = [[None] * c.P for _ in range(c.NC)]
    for k in range(c.NC):
        for p in range(c.P):
            tt = len(pr.tiles2[p])
            oh2 = np.zeros((128, tt, 2, 128), bf)
            icv = np.zeros((128, tt), np.float32)
            per_h = [np.zeros(max(pr.htiles2[p][h], 1) * 128, np.int64)
                     for h in range(c.NH)]
            consumed = {}
            for gti, (b, h, posh) in enumerate(pr.tiles2[p]):
                s_arr, d_arr, ic_arr = l2_lists[k][p][b][h]
                off = consumed.get((b, h), 0)
                sl = s_arr[off:off + 128]
                dl = d_arr[off:off + 128]
                il = ic_arr[off:off + 128]
                consumed[(b, h)] = off + 128
                n = len(sl)
                per_h[h][posh * 128:posh * 128 + n] = dl
                q = np.arange(n)
                oh2[q, gti, 0, sl] = 1.0      # oh[slot, src_local]
                oh2[sl, gti, 1, q] = 1.0      # oht[src_local, slot]
                icv[:n, pr.hoff[p][h] + posh] = il
            for h in range(c.NH):
                pr.gidx2d[k][p][h] = wrap_idx(per_h[h])
            pr.oh2[k][p] = oh2
            pr.icvt[k][p] = icv

    # ---------------- weights packing ------------------------------------
    F = c.F
    nex_w1 = np.asarray(inputs["nex_w1"], np.float32)
    nex_b1 = np.asarray(inputs["nex_b1"], np.float32)
    nex_w2 = np.asarray(inputs["nex_w2"], np.float32)
    nex_b2 = np.asarray(inputs["nex_b2"], np.float32)
    edge_w1 = np.asarray(inputs["edge_w1"], np.float32)
    edge_b1 = np.asarray(inputs["edge_b1"], np.float32)
    edge_w2 = np.asarray(inputs["edge_w2"], np.float32)
    edge_b2 = np.asarray(inputs["edge_b2"], np.float32)
    node_w1 = np.asarray(inputs["node_w1"], np.float32)
    node_b1 = np.asarray(inputs["node_b1"], np.float32)
    node_w2 = np.asarray(inputs["node_w2"], np.float32)
    node_b2 = np.asarray(inputs["node_b2"], np.float32)

    G = c.GRP
    W = {}
    for gi, g in enumerate(G):
        for p in range(c.P):
            W[f"nexW1_{p}_{gi}"] = to_bf16(
                blkdiag([nex_w1[cc, p * F:(p + 1) * F, :] for cc in g]))
            W[f"anW_{p}_{gi}"] = to_bf16(
                blkdiag([edge_w1[p, cc, F:, :] for cc in g]))
            W[f"zW_{p}_{gi}"] = to_bf16(
                blkdiag([node_w1[p, cc, F:, :] for cc in g]))
            # axnW padded to full 128 so agg seed rows are fully written
            W[f"axnW_{p}_{gi}"] = to_bf16(
                blkdiag([node_w1[p, cc, :F, :] for cc in g],
                        rows=128, cols=128))
            W[f"u2W_{p}_{gi}"] = to_bf16(
                blkdiag([node_w2[p, cc] for cc in g]))
            W[f"anB_{p}_{gi}"] = np.concatenate(
                [edge_b1[p, cc] for cc in g])[:, None].astype(np.float32)
            axnb = np.zeros((128, 1), np.float32)
            nb_cat = np.concatenate([node_b1[p, cc] for cc in g])
            axnb[:len(nb_cat), 0] = nb_cat
            W[f"axnB_{p}_{gi}"] = axnb
            W[f"u2B_{p}_{gi}"] = np.concatenate(
                [node_b2[p, cc] for cc in g])[:, None].astype(np.float32)
        W[f"nexW2_{gi}"] = to_bf16(blkdiag([nex_w2[cc] for cc in g]))
        W[f"nexB1_{gi}"] = np.concatenate(
            [nex_b1[cc] for cc in g])[:, None].astype(np.float32)
        W[f"nexB2_{gi}"] = np.concatenate(
            [nex_b2[cc] for cc in g])[:, None].astype(np.float32)
    for p in range(c.P):
        # ax weights: full class-blkdiag [384, 320] in 3 row chunks of 128
        axw = blkdiag([edge_w1[p, cc, :F, :] for cc in range(c.C)],
                      rows=384, cols=320)
        W[f"axW_{p}"] = to_bf16(axw.reshape(3, 128, 320)
                                .transpose(1, 0, 2).copy())
        W[f"w2rep_{p}"] = np.tile(
            to_bf16(edge_w2[p, :, :, 0].reshape(1, c.C * c.EF)), (128, 1))
        W[f"b2rep_{p}"] = np.tile(edge_b2[p, :, 0].reshape(1, 1, c.C),
                                  (128, c.W2B, 1)).astype(np.float32)
    W["idbf"] = to_bf16(np.eye(128, dtype=np.float32))
    pr.W = W
    return pr


# ----------------------------------------------------------------------------
# Launch 1 builder (nexus phase)
# ----------------------------------------------------------------------------

def build_l1(cfg, pr):
    c = cfg
    nc = bacc.Bacc("TRN2", target_bir_lowering=False, debug=False,
                   num_devices=c.NC)

    xg = [nc.dram_tensor(f"xg{p}", [128, pr.T1tot[p], 320], F8,
                         kind="ExternalInput") for p in range(c.P)]
    oh1 = [nc.dram_tensor(f"oh1_{p}", [128, pr.T1tot[p], 128], F8,
                          kind="ExternalInput") for p in range(c.P)]

    wnames = ["idbf"]
    for gi in range(len(c.GRP)):
        wnames += [f"nexW2_{gi}", f"nexB1_{gi}", f"nexB2_{gi}"]
        for p in range(c.P):
            wnames += [f"nexW1_{p}_{gi}", f"anW_{p}_{gi}", f"zW_{p}_{gi}",
                       f"anB_{p}_{gi}"]
    wt = {n: nc.dram_tensor(n, list(pr.W[n].shape), _wdt(pr.W[n]),
                            kind="ExternalInput") for n in wnames}

    andz = nc.dram_tensor("andz", [c.P, c.MKP, c.RZ], BF,
                          kind="ExternalOutput")

    G = c.GRP
    # tile start offset per (p, w)
    toff = [[0] * (c.NW + 1) for _ in range(c.P)]
    for p in range(c.P):
        for w in range(c.NW):
            toff[p][w + 1] = toff[p][w] + pr.T1[p][w]
    ntmax = max(toff[p][min(s * c.SLW + c.SLW, c.NW)] - toff[p][s * c.SLW]
                for p in range(c.P) for s in range(c.NSL1))

    with tile.TileContext(nc) as tc:
        with tc.tile_pool(name="const", bufs=1) as cpool, \
             tc.tile_pool(name="xgp", bufs=3) as xgpool, \
             tc.tile_pool(name="ohp", bufs=3) as ohpool, \
             tc.tile_pool(name="awp", bufs=3) as awpool, \
             tc.tile_pool(name="wrk", bufs=4) as wpool, \
             tc.tile_pool(name="rowp", bufs=4) as rpool, \
             tc.tile_pool(name="psA", bufs=2, space="PSUM") as psA, \
             tc.tile_pool(name="psM", bufs=1, space="PSUM") as psM, \
             tc.tile_pool(name="psT", bufs=2, space="PSUM") as psT:

            cw = {}
            for n in wnames:
                t = cpool.tile(list(pr.W[n].shape), _wdt(pr.W[n]), tag=n)
                nc.sync.dma_start(out=t[:], in_=wt[n].ap())
                cw[n] = t

            for sl in range(c.NSL1):
                w0 = sl * c.SLW
                w1 = min(w0 + c.SLW, c.NW)
                nw = w1 - w0
                awT = {}
                # ---- phase A: feature-major window sums ----
                for p in range(c.P):
                    t0, t1 = toff[p][w0], toff[p][w1]
                    nt = t1 - t0
                    xt = xgpool.tile([128, ntmax, 320], F8, tag=f"xg{p}",
                                     name=f"xg_{sl}_{p}")
                    nc.sync.dma_start(out=xt[:, :nt, :],
                                      in_=xg[p].ap()[:, t0:t1, :])
                    oht = ohpool.tile([128, ntmax, 128], F8, tag=f"oh{p}",
                                      name=f"oh_{sl}_{p}")
                    nc.sync.dma_start(out=oht[:, :nt, :],
                                      in_=oh1[p].ap()[:, t0:t1, :])
                    aw = awpool.tile([128, 3, 128 * c.SLW], BF, tag=f"awT{p}",
                                     name=f"awT_{sl}_{p}")
                    awT[p] = aw
                    for w in range(w0, w1):
                        wl = w - w0
                        ntw = pr.T1[p][w]
                        ps = psA.tile([128, 3, 128], F32, tag="psA")
                        # PSUM start marks the whole 2KB bank pending-zero:
                        # only the FIRST matmul into the tile may set start.
                        for j in range(ntw):
                            lt = toff[p][w] - t0 + j
                            for fj, fl in enumerate((128, 128, 64)):
                                nc.tensor.matmul(
                                    out=ps[:fl, fj, :],
                                    lhsT=xt[:, lt, fj * 128:fj * 128 + fl],
                                    rhs=oht[:, lt, :],
                                    start=(j == 0 and fj == 0),
                                    stop=(j == ntw - 1 and fj == 2))
                        nc.scalar.activation(
                            out=aw[:, :2, wl * 128:(wl + 1) * 128],
                            in_=ps[:, :2, :], func=AF.Copy)
                        nc.scalar.activation(
                            out=aw[:64, 2, wl * 128:(wl + 1) * 128],
                            in_=ps[:64, 2, :], func=AF.Copy)

                # ---- phase B: nexus MLP over the slab, feature-major ----
                wid = 128 * nw
                ntt = []
                for gi, g in enumerate(G):
                    gp = c.GP[gi]
                    m1 = psM.tile([128, 128 * c.SLW], F32, tag="m1")
                    for p in range(c.P):
                        nc.tensor.matmul(out=m1[:gp, :wid],
                                         lhsT=cw[f"nexW1_{p}_{gi}"][:],
                                         rhs=awT[p][:gp, gi, :wid],
                                         start=(p == 0), stop=(p == c.P - 1))
                    h1 = wpool.tile([128, 128 * c.SLW], BF, tag="h1")
                    nc.scalar.activation(out=h1[:gp, :wid], in_=m1[:gp, :wid],
                                         func=AF.Tanh,
                                         bias=cw[f"nexB1_{gi}"][:gp, :])
                    m2 = psM.tile([128, 128 * c.SLW], F32, tag="m2")
                    nc.tensor.matmul(out=m2[:gp, :wid],
                                     lhsT=cw[f"nexW2_{gi}"][:],
                                     rhs=h1[:gp, :wid], start=True, stop=True)
                    nt = wpool.tile([128, 128 * c.SLW], BF, tag=f"nt{gi}")
                    nc.scalar.activation(out=nt[:gp, :wid], in_=m2[:gp, :wid],
                                         func=AF.Tanh,
                                         bias=cw[f"nexB2_{gi}"][:gp, :])
                    ntt.append(nt)

                for p in range(c.P):
                    anz = wpool.tile([128, 6, 128 * c.SLW], BF, tag="anz")
                    for hi, wkey in ((0, "anW"), (1, "zW")):
                        for gi, g in enumerate(G):
                            gp = c.GP[gi]
                            mm = psM.tile([128, 128 * c.SLW], F32,
                                          tag="anzm", bufs=2)
                            nc.tensor.matmul(out=mm[:gp, :wid],
                                             lhsT=cw[f"{wkey}_{p}_{gi}"][:],
                                             rhs=ntt[gi][:gp, :wid],
                                             start=True, stop=True)
                            if hi == 0:
                                nc.scalar.activation(
                                    out=anz[:gp, hi * 3 + gi, :wid],
                                    in_=mm[:gp, :wid], func=AF.Identity,
                                    bias=cw[f"anB_{p}_{gi}"][:gp, :])
                            else:
                                nc.vector.tensor_copy(
                                    out=anz[:gp, hi * 3 + gi, :wid],
                                    in_=mm[:gp, :wid])
                    for w in range(w0, w1):
                        wl = w - w0
                        pt = psT.tile([128, c.RZ], BF, tag="psT")
                        for hi in range(2):
                            for gi in range(3):
                                gp = c.GP[gi]
                                off = hi * 320 + gi * 128
                                nc.tensor.transpose(
                                    out=pt[:, off:off + gp],
                                    in_=anz[:gp, hi * 3 + gi,
                                            wl * 128:(wl + 1) * 128],
                                    identity=cw["idbf"][:gp, :gp])
                        row = rpool.tile([128, c.RZ], BF, tag="row")
                        nc.vector.tensor_copy(out=row[:], in_=pt[:])
                        nc.sync.dma_start(
                            out=andz.ap()[p, w * 128:(w + 1) * 128, :],
                            in_=row[:])
    nc.compile()
    innames = ([f"xg{p}" for p in range(c.P)]
               + [f"oh1_{p}" for p in range(c.P)] + wnames)
    return nc, innames


# ----------------------------------------------------------------------------
# Launch 2 builder (edge + node phase)
# ----------------------------------------------------------------------------

GATHER_MAX_TILES = 8   # 1024 idxs: SWDGE descriptor ring holds only 1024


def build_l2(cfg, pr):
    c = cfg
    nc = bacc.Bacc("TRN2", target_bir_lowering=False, debug=False,
                   num_devices=c.NC)

    xt_d = [nc.dram_tensor(f"xt{p}", [128, 3, c.NKP512], BF,
                           kind="ExternalInput") for p in range(c.P)]
    andz = nc.dram_tensor("andz", [c.P, c.HROWS, c.RZ], BF,
                          kind="ExternalInput")
    g2d = [[nc.dram_tensor(f"g2d{p}_{h}", list(pr.gidx2d[0][p][h].shape), I16,
                           kind="ExternalInput") for h in range(c.NH)]
           for p in range(c.P)]
    oh2 = [nc.dram_tensor(f"oh2_{p}", [128, len(pr.tiles2[p]), 2, 128], BF,
                          kind="ExternalInput") for p in range(c.P)]
    icvt = [nc.dram_tensor(f"icvt{p}", [128, len(pr.tiles2[p])], F32,
                           kind="ExternalInput") for p in range(c.P)]

    wnames = ["idbf"]
    for p in range(c.P):
        wnames += [f"axW_{p}", f"w2rep_{p}", f"b2rep_{p}"]
        for gi in range(len(c.GRP)):
            wnames += [f"axnW_{p}_{gi}", f"u2W_{p}_{gi}",
                       f"axnB_{p}_{gi}", f"u2B_{p}_{gi}"]
    wt = {n: nc.dram_tensor(n, list(pr.W[n].shape), _wdt(pr.W[n]),
                            kind="ExternalInput") for n in wnames}

    out = nc.dram_tensor("out", [c.P, c.NUSL, 320, c.UB * 128], BF,
                         kind="ExternalOutput")

    G = c.GRP
    hoff = pr.hoff
    # per (p, b): ordered list of (gti, h, posh); plus global tile offsets
    btiles = [[[] for _ in range(c.NB)] for _ in range(c.P)]
    for p in range(c.P):
        for gti, (b, h, posh) in enumerate(pr.tiles2[p]):
            btiles[p][b].append((gti, h, posh))
    # max one-hot slab width over (p, g) for uniform tile shapes
    ngtmax = 1
    for p in range(c.P):
        for g in range(pr.NBG):
            b0, b1 = g * c.BG, min((g + 1) * c.BG, c.NB)
            tl = [te for b in range(b0, b1) for te in btiles[p][b]]
            if tl:
                ngtmax = max(ngtmax, tl[-1][0] + 1 - tl[0][0])

    with tile.TileContext(nc) as tc:
        with tc.tile_pool(name="const", bufs=1) as cpool, \
             tc.tile_pool(name="gd", bufs=5) as gdpool, \
             tc.tile_pool(name="ohp", bufs=3) as ohpool, \
             tc.tile_pool(name="xtp", bufs=3) as xtpool, \
             tc.tile_pool(name="axp", bufs=3) as axpool, \
             tc.tile_pool(name="wrk", bufs=4) as wpool, \
             tc.tile_pool(name="u1p", bufs=3) as u1pool, \
             tc.tile_pool(name="u2p", bufs=3) as u2pool, \
             tc.tile_pool(name="psE", bufs=2, space="PSUM") as psE, \
             tc.tile_pool(name="psG", bufs=2, space="PSUM") as psG, \
             tc.tile_pool(name="psP", bufs=1, space="PSUM") as psP:
            psU = psP

            cw = {}
            for n in wnames:
                t = cpool.tile(list(pr.W[n].shape), _wdt(pr.W[n]), tag=n)
                nc.sync.dma_start(out=t[:], in_=wt[n].ap())
                cw[n] = t
            cs = {}
            for p in range(c.P):
                for h in range(c.NH):
                    t = cpool.tile(list(pr.gidx2d[0][p][h].shape), I16,
                                   tag=f"g2d{p}{h}")
                    nc.sync.dma_start(out=t[:], in_=g2d[p][h].ap())
                    cs[(p, h)] = t
                t = cpool.tile([128, len(pr.tiles2[p])], F32, tag=f"ic{p}")
                nc.sync.dma_start(out=t[:], in_=icvt[p].ap())
                cs[("i", p)] = t

            for p in range(c.P):
                u1 = None
                for g in range(pr.NBG):
                    b0 = g * c.BG
                    b1 = min(b0 + c.BG, c.NB)
                    # ---- load xT slab, one-hot slab ----
                    xt = xtpool.tile([128, 3, c.BG * 128], BF, tag="xt",
                                     name=f"xt_{p}_{g}")
                    nc.sync.dma_start(
                        out=xt[:, :, :(b1 - b0) * 128],
                        in_=xt_d[p].ap()[:, :, b0 * 128:b1 * 128])
                    tlist = [te for b in range(b0, b1) for te in btiles[p][b]]
                    gt0 = tlist[0][0] if tlist else 0
                    gt1 = tlist[-1][0] + 1 if tlist else 0
                    ngt = gt1 - gt0
                    ohs = ohpool.tile([128, ngtmax, 2, 128], BF,
                                      tag="ohs", name=f"ohs_{p}_{g}")
                    if ngt:
                        nc.sync.dma_start(
                            out=ohs[:, :ngt, :, :],
                            in_=oh2[p].ap()[:, gt0:gt1, :, :])

                    # ---- gathers for this group: sub-calls aligned to SG
                    # boundaries so softmax batches never straddle tiles ----
                    gsub = {}
                    for h in range(c.NH):
                        dt0, dn = pr.seg2d[p][h][g]
                        chunks = []
                        pos = dt0
                        for s0 in range(b0, b1, c.SG):
                            cnt = sum(pr.T2[p][b][h]
                                      for b in range(s0,
                                                     min(s0 + c.SG, b1)))
                            done = 0
                            while done < cnt:
                                n = min(GATHER_MAX_TILES, cnt - done)
                                gtile = gdpool.tile(
                                    [128, GATHER_MAX_TILES, c.RZ], BF,
                                    tag="gd",
                                    name=f"gd_{p}_{g}_{h}_{pos + done}")
                                nc.gpsimd.dma_gather(
                                    out_ap=gtile[:, :n, :],
                                    in_ap=andz.ap()[p, h * c.DCH:
                                                    min((h + 1) * c.DCH,
                                                        c.HROWS), :],
                                    idxs_ap=cs[(p, h)][:,
                                                       (pos + done) * 8:
                                                       (pos + done + n) * 8],
                                    num_idxs=n * 128,
                                    num_idxs_reg=n * 128,
                                    elem_size=c.RZ,
                                )
                                chunks.append((pos + done, n, gtile))
                                done += n
                            pos += cnt
                        gsub[h] = chunks

                    # ---- prologue: ax (node-major) + axnT (feature-major)
                    axg = axpool.tile([128, c.BG, 320], BF, tag="axg",
                                      name=f"axg_{p}_{g}")
                    for b in range(b0, b1):
                        pp = psP.tile([128, 320], F32, tag="axg")
                        for j in range(3):
                            nc.tensor.matmul(
                                out=pp[:],
                                lhsT=xt[:, j, (b - b0) * 128:
                                        (b - b0 + 1) * 128],
                                rhs=cw[f"axW_{p}"][:, j, :],
                                start=(j == 0), stop=(j == 2))
                        nc.vector.tensor_copy(out=axg[:, b - b0, :],
                                              in_=pp[:])
                    axnT = axpool.tile([128, 3, c.BG * 128], BF,
                                       tag="axnT", name=f"axnT_{p}_{g}")
                    for ci in range(ceil_div((b1 - b0) * 128, 512)):
                        c0 = ci * 512
                        cl = min(512, (b1 - b0) * 128 - c0)
                        for gi in range(3):
                            pp = psP.tile([128, 512], F32, tag="p512")
                            nc.tensor.matmul(
                                out=pp[:, :cl],
                                lhsT=cw[f"axnW_{p}_{gi}"][:],
                                rhs=xt[:128, gi, c0:c0 + cl],
                                start=True, stop=True)
                            nc.scalar.activation(
                                out=axnT[:, gi, c0:c0 + cl],
                                in_=pp[:, :cl], func=AF.Identity,
                                bias=cw[f"axnB_{p}_{gi}"][:, :])

                    # ---- edges: sub-groups of SG blocks ----
                    for sg0 in range(b0, b1, c.SG):
                        sg1 = min(sg0 + c.SG, b1)
                        # two blocks per 2-bank PSUM tile [128, 3, 256]:
                        # region j holds feature-chunk j for both blocks.
                        # j==2's bytes live in the second bank, so it also
                        # needs start=True (bank-level pending-zero mark).
                        aggs = {}
                        pairlast = {}
                        for b in range(sg0, sg1):
                            if (b - sg0) % 2 == 0:
                                pair = psG.tile([128, 3, 256], F32,
                                                tag="agg",
                                                name=f"agg_{p}_{b}")
                                tl2 = btiles[p][b] + btiles[p][b + 1]
                                plast = (max(te[0] for te in tl2)
                                         if tl2 else -1)
                                # seed from axnT (identity matmul)
                                for j in range(3):
                                    nc.tensor.matmul(
                                        out=pair[:, j, :],
                                        lhsT=cw["idbf"][:],
                                        rhs=axnT[:, j, (b - b0) * 128:
                                                 (b - b0 + 2) * 128],
                                        start=(j in (0, 2)),
                                        stop=(plast < 0 and j == 2))
                            aggs[b] = (pair, ((b - sg0) % 2) * 128)
                            pairlast[b] = plast
                        # batches: consecutive tiles of one (h, sub-gather)
                        tl = [te for b in range(sg0, sg1)
                              for te in btiles[p][b]]
                        tl.sort(key=lambda te: te[0])   # global order
                        byh = {}
                        for te in tl:
                            byh.setdefault(te[1], []).append(te)
                        for h in sorted(byh):
                            tes = byh[h]
                            i = 0
                            while i < len(tes):
                                # batch limited to one sub-gather tile
                                gti0, _, posh0 = tes[i]
                                cst = cn = gtile = None
                                for cst, cn, gtile in gsub[h]:
                                    if cst <= posh0 < cst + cn:
                                        break
                                nb = 1
                                while (nb < c.W2B and i + nb < len(tes)
                                       and tes[i + nb][2] == posh0 + nb
                                       and posh0 + nb < cst + cn):
                                    nb += 1
                                batch = tes[i:i + nb]
                                i += nb
                                j0 = posh0 - cst
                                adz = gtile[:, j0:j0 + nb, :]
                                # axe: select ax rows + add gathered an;
                                # per-tile PSUM (1 bank), tanh into a slab
                                ht = wpool.tile([128, c.W2B, 320], BF,
                                                tag="ht")
                                for jj, (gti, _, _) in enumerate(batch):
                                    bb = pr.tiles2[p][gti][0]
                                    pe = psE.tile([128, 320], F32, tag="pe")
                                    nc.tensor.matmul(
                                        out=pe[:],
                                        lhsT=ohs[:, gti - gt0, 1, :],
                                        rhs=axg[:, bb - b0, :],
                                        start=True, stop=False)
                                    nc.tensor.matmul(
                                        out=pe[:],
                                        lhsT=cw["idbf"][:],
                                        rhs=adz[:, jj, :320],
                                        start=False, stop=True)
                                    nc.scalar.activation(out=ht[:, jj, :],
                                                         in_=pe[:],
                                                         func=AF.Tanh)
                                lm = wpool.tile([128, c.W2B, 320], BF,
                                                tag="lm")
                                nc.vector.tensor_tensor(
                                    out=lm[:, :nb, :], in0=ht[:, :nb, :],
                                    in1=cw[f"w2rep_{p}"][:]
                                    .rearrange("q (o f) -> q o f", o=1)
                                    .to_broadcast([128, nb, 320]),
                                    op=OP.mult)
                                lg = wpool.tile([128, c.W2B, c.C], BF,
                                                tag="lg")
                                with nc.allow_low_precision(
                                        reason="f32 accum, bf16 store"):
                                    nc.vector.tensor_reduce(
                                        out=lg[:, :nb, :],
                                        in_=lm[:, :nb, :]
                                        .rearrange("q a (c f) -> q a c f",
                                                   f=64),
                                        axis=mybir.AxisListType.X, op=OP.add)
                                ex = wpool.tile([128, c.W2B, c.C], F32,
                                                tag="ex")
                                nc.gpsimd.tensor_add(
                                    out=ex[:, :nb, :],
                                    in0=lg[:, :nb, :],
                                    in1=cw[f"b2rep_{p}"][:, :nb, :])
                                exe = wpool.tile([128, c.W2B, c.C], F32,
                                                 tag="exe")
                                nc.scalar.activation(out=exe[:, :nb, :],
                                                     in_=ex[:, :nb, :],
                                                     func=AF.Exp)
                                den = wpool.tile([128, c.W2B], F32,
                                                 tag="den")
                                nc.vector.tensor_reduce(
                                    out=den[:, :nb], in_=exe[:, :nb, :],
                                    axis=mybir.AxisListType.X, op=OP.add)
                                rec = wpool.tile([128, c.W2B], F32,
                                                 tag="rec")
                                nc.vector.reciprocal(out=rec[:, :nb],
                                                     in_=den[:, :nb])
                                hcol0 = hoff[p][h] + posh0
                                wsc = wpool.tile([128, c.W2B], F32,
                                                 tag="wsc")
                                nc.vector.tensor_tensor(
                                    out=wsc[:, :nb], in0=rec[:, :nb],
                                    in1=cs[("i", p)][:, hcol0:hcol0 + nb],
                                    op=OP.mult)
                                ws = wpool.tile([128, c.W2B, c.C], BF,
                                                tag="ws")
                                nc.vector.tensor_tensor(
                                    out=ws[:, :nb, :], in0=exe[:, :nb, :],
                                    in1=wsc[:, :nb]
                                    .rearrange("q (a o) -> q a o", o=1)
                                    .to_broadcast([128, nb, c.C]),
                                    op=OP.mult)
                                msg = wpool.tile([128, c.W2B, 320], BF,
                                                 tag="msg")
                                nc.vector.tensor_tensor(
                                    out=msg[:, :nb, :]
                                    .rearrange("q a (c f) -> q a c f", f=64),
                                    in0=adz[:, :nb, 320:]
                                    .rearrange("q a (c f) -> q a c f", f=64),
                                    in1=ws[:, :nb, :]
                                    .rearrange("q a (c o) -> q a c o", o=1)
                                    .to_broadcast([128, nb, c.C, 64]),
                                    op=OP.mult)
                                for jj, (gti, hh, _) in enumerate(batch):
                                    bb = pr.tiles2[p][gti][0]
                                    last = (gti == pairlast[bb])
                                    agg, boff = aggs[bb]
                                    for fj, fl in enumerate((128, 128, 64)):
                                        nc.tensor.matmul(
                                            out=agg[:fl, fj,
                                                    boff:boff + 128],
                                            lhsT=msg[:, jj, fj * 128:
                                                     fj * 128 + fl],
                                            rhs=ohs[:, gti - gt0, 0, :],
                                            start=False,
                                            stop=(last and fj == 2))
                        # ---- u1 per block pair; u2 per UB-slab ----
                        for b in range(sg0, sg1):
                            if b % c.UB == 0:
                                u1 = u1pool.tile([128, 3, c.UB * 128], BF,
                                                 tag="u1")
                            if (b - sg0) % 2 == 1:
                                pair = aggs[b][0]
                                bl = (b - 1) % c.UB
                                nc.scalar.activation(
                                    out=u1[:, :, bl * 128:(bl + 2) * 128],
                                    in_=pair[:], func=AF.Tanh)
                            if b % c.UB == c.UB - 1:
                                sli = b // c.UB
                                for gi in range(3):
                                    gp = c.GP[gi]
                                    mu = psU.tile([128, c.UB * 128], F32,
                                                  tag="p512")
                                    nc.tensor.matmul(
                                        out=mu[:gp, :],
                                        lhsT=cw[f"u2W_{p}_{gi}"][:],
                                        rhs=u1[:gp, gi, :],
                                        start=True, stop=True)
                                    u2f = u2pool.tile([128, c.UB * 128], BF,
                                                      tag="u2f")
                                    nc.scalar.activation(
                                        out=u2f[:gp, :], in_=mu[:gp, :],
                                        func=AF.Tanh,
                                        bias=cw[f"u2B_{p}_{gi}"][:gp, :])
                                    nc.sync.dma_start(
                                        out=out.ap()[p, sli,
                                                     gi * 128:gi * 128 + gp,
                                                     :],
                                        in_=u2f[:gp, :])
    nc.compile()
    innames = ([f"xt{p}" for p in range(c.P)] + ["andz"]
               + [f"g2d{p}_{h}" for p in range(c.P) for h in range(c.NH)]
               + [f"oh2_{p}" for p in range(c.P)]
               + [f"icvt{p}" for p in range(c.P)] + wnames)
    return nc, innames


# ----------------------------------------------------------------------------
# in_maps
# ----------------------------------------------------------------------------

def l1_inmaps(cfg, pr, names):
    c = cfg
    maps = []
    for k in range(c.NC):
        m = {}
        for p in range(c.P):
            m[f"xg{p}"] = pr.xg[k][p]
            m[f"oh1_{p}"] = pr.oh1[k][p]
        for n in pr.W:
            m[n] = pr.W[n]
        maps.append({n: m[n] for n in names})
    return maps


def l2_inmaps(cfg, pr, andz_full, names):
    c = cfg
    maps = []
    for k in range(c.NC):
        m = {"andz": andz_full}
        for p in range(c.P):
            m[f"xt{p}"] = pr.xt[k][p]
            for h in range(c.NH):
                m[f"g2d{p}_{h}"] = pr.gidx2d[k][p][h]
            m[f"oh2_{p}"] = pr.oh2[k][p]
            m[f"icvt{p}"] = pr.icvt[k][p]
        for n in pr.W:
            m[n] = pr.W[n]
        maps.append({n: m[n] for n in names})
    return maps


# ----------------------------------------------------------------------------
# public kernel()
# ----------------------------------------------------------------------------

def run(cfg, inputs, runner=None):
    """runner(nc, maps) -> list of per-core output dicts; default = HW SPMD."""
    pr = host_prep(cfg, inputs)

    nc1, in1 = build_l1(cfg, pr)
    maps1 = l1_inmaps(cfg, pr, in1)
    if runner is None:
        res1 = run_bass_kernel_spmd(nc1, maps1,
                                    core_ids=list(range(cfg.NC))).results
    else:
        res1 = runner(nc1, maps1)
    andz_full = np.concatenate(
        [np.asarray(res1[k]["andz"]) for k in range(cfg.NC)], axis=1)

    nc2, in2 = build_l2(cfg, pr)
    maps2 = l2_inmaps(cfg, pr, andz_full, in2)
    if runner is None:
        res2 = run_bass_kernel_spmd(nc2, maps2,
                                    core_ids=list(range(cfg.NC))).results
    else:
        res2 = runner(nc2, maps2)

    parts = []
    for k in range(cfg.NC):
        o = np.asarray(res2[k]["out"]).astype(np.float32)
        # [P, NUSL, 320, UB*128] -> [P, nodes, 320]
        o = np.transpose(o, (0, 1, 3, 2)).reshape(cfg.P, cfg.NKP, 320)
        parts.append(o[:, :cfg.NK, :])
    out = np.concatenate(parts, axis=1)
    return np.ascontiguousarray(
        out.reshape(cfg.P, cfg.N, cfg.C, cfg.F).astype(np.float32))


def kernel(**inputs):
    return run(Cfg(), inputs)

